# revision 1
# baseline (speedup 1.0000x reference)
"""ArDCA loss on 8 TRN2 NeuronCores, data-parallel over M.

Per core (M/8 = 1024 sequences):
  - build one-hot H^T (5418 x 1024, fp8 pair-layout) on device from seqs via
    a replication matmul + is_equal compare
  - contrib^T = W^T @ H^T as causal-masked fp8 DoubleRow matmuls on TensorE
    (W scaled x16, pre-transposed/masked/paired on host, streamed from DRAM)
  - logits^T handled implicitly: E = exp(P/16 + h) fused on ScalarE; selected
    -logit and log-Z pieces accumulated via VectorE + small ones-matmuls
  - per-core partial scalars [U_P(scaled), sum_w] DMA'd out
Host combines partials with the exact-f32 h-gather and regularizer sums.
"""

import os
import sys

for _p in ("/opt/trn_rl_repo",):
    if _p not in sys.path:
        sys.path.append(_p)

from contextlib import ExitStack

import numpy as np
import ml_dtypes

import concourse.bass as bass
import concourse.mybir as mybir
from concourse import tile
from concourse.bass_utils import run_bass_kernel_spmd

BF16 = ml_dtypes.bfloat16
F8 = ml_dtypes.float8_e4m3
FP32 = mybir.dt.float32
BF = mybir.dt.bfloat16
FP8 = mybir.dt.float8e4

L, Q, M, NC = 256, 21, 8192, 8
LT, TK, T = 258, 126, 43  # padded L, tile width (6*21), tile count
NU = (T + 1) // 2  # number of H^T pair tiles (22)
MS = M // NC
JK = LT * Q
NPAIRS = sum((t + 2) // 2 for t in range(T))  # 484
SCALE = 16.0
LAMBDA_H, LAMBDA_J = 1e-06, 1e-4
AF = mybir.ActivationFunctionType
OP = mybir.AluOpType


def _legalize_waits(nc):
    """Split >cap sync waits into preceding EventSemaphore instructions.

    This container's walrus accepts at most 1 wait per instruction (2 on
    EventSemaphore); Tile's final drain carries one wait per used processor.
    """
    n_split = 0
    for f in nc.m.functions:
        for bb in f.blocks:
            if not any(
                ins.sync_info
                and ins.sync_info.on_wait
                and len(ins.sync_info.on_wait)
                > (2 if isinstance(ins, mybir.InstEventSemaphore) else 1)
                for ins in bb.instructions
            ):
                continue
            new_list = []
            for ins in bb.instructions:
                si = ins.sync_info
                waits = list(si.on_wait) if si and si.on_wait else []
                cap = 2 if isinstance(ins, mybir.InstEventSemaphore) else 1
                if len(waits) > cap:
                    extra, keep = waits[:-cap], waits[-cap:]
                    for k in range(0, len(extra), 2):
                        ev = mybir.InstEventSemaphore(
                            name=f"EVSPLIT-{n_split}", ins=[], outs=[]
                        )
                        n_split += 1
                        ev.engine = ins.engine
                        ev.sync_info = mybir.SyncInfo(
                            on_wait=extra[k : k + 2], on_update=[]
                        )
                        new_list.append(ev)
                        nc.register_instruction(ev, overwrite=True)
                    si.on_wait = keep
                new_list.append(ins)
            try:
                bb.instructions = new_list
            except Exception:
                bb.instructions.clear()
                bb.instructions.extend(new_list)
    return n_split


def build_nc():
    nc = bass.Bass()
    wt_e = nc.declare_dram_parameter("wt", [NPAIRS, TK, 2, 128], FP8, isOutput=False)
    hp_e = nc.declare_dram_parameter("hp", [NU, TK, 2, MS], FP8, isOutput=False)
    ht_e = nc.declare_dram_parameter("ht", [T, TK], FP32, isOutput=False)
    w_e = nc.declare_dram_parameter("w", [1, MS], FP32, isOutput=False)
    hv_e = nc.declare_dram_parameter("hv", [1, MS], FP32, isOutput=False)
    on6_e = nc.declare_dram_parameter("on6", [TK, 6], BF, isOutput=False)
    o6f_e = nc.declare_dram_parameter("o6f", [6, 1], FP32, isOutput=False)
    o126_e = nc.declare_dram_parameter("o126", [TK, 1], FP32, isOutput=False)
    out_e = nc.declare_dram_parameter("out", [1, 4], FP32, isOutput=True)

    with tile.TileContext(nc) as tc, ExitStack() as ctx:
        cpool = ctx.enter_context(tc.tile_pool(name="const", bufs=1))
        htp = ctx.enter_context(tc.tile_pool(name="htp", bufs=1))
        wpool = ctx.enter_context(tc.tile_pool(name="wp", bufs=2))
        sqp = ctx.enter_context(tc.tile_pool(name="sqp", bufs=3))
        ep = ctx.enter_context(tc.tile_pool(name="ep", bufs=2))
        accp = ctx.enter_context(tc.tile_pool(name="accp", bufs=1))
        pbufs = int(os.environ.get("KT_PBUFS", "2"))
        pP = ctx.enter_context(tc.tile_pool(name="pP", bufs=pbufs, space="PSUM"))
        pZ = ctx.enter_context(
            tc.tile_pool(name="pZ", bufs=4 - pbufs, space="PSUM")
        )

        # constants
        on6 = cpool.tile([TK, 6], BF)
        nc.gpsimd.dma_start(on6[:], on6_e[:])
        o6f = cpool.tile([6, 1], FP32)
        nc.gpsimd.dma_start(o6f[:], o6f_e[:])
        o126 = cpool.tile([TK, 1], FP32)
        nc.gpsimd.dma_start(o126[:], o126_e[:])
        htt = cpool.tile([TK, T], FP32)
        nc.gpsimd.dma_start(htt[:], ht_e[:].rearrange("t p -> p t"))
        wv = cpool.tile([1, MS], FP32)
        nc.gpsimd.dma_start(wv[:], w_e[:])
        hv = cpool.tile([1, MS], FP32)
        nc.gpsimd.dma_start(hv[:], hv_e[:])

        # accumulators
        accS = accp.tile([TK, MS], FP32)
        nc.vector.memset(accS[:], 0.0)
        accZ = accp.tile([6, MS], FP32)
        nc.vector.memset(accZ[:], 0.0)

        # ---- phase B: one-hot pair tiles, DMA'd just-in-time on the
        # scalar HWDGE ring (separate FIFO from the W-strip DMAs) ----
        hps = [
            htp.tile([TK, 2, MS], FP8, tag=f"hp{u}", name=f"hp{u}") for u in range(NU)
        ]
        hp_issued = set()

        def need_hps(tt):
            for u in range((tt + 2) // 2):
                if u not in hp_issued:
                    hp_issued.add(u)
                    nc.scalar.dma_start(hps[u][:], hp_e[u])

        # ---- phase C: main causal DoubleRow matmul loop ----
        # zigzag order around the median tile so per-step PE work is roughly
        # constant and never gated by the fixed per-tile epilogue cost
        t_lim = int(os.environ.get("KT_LIM", T))
        if os.environ.get("KT_ZIGZAG", "0") == "1":
            order = []
            lo, hi = T // 2, T // 2 + 1
            while lo >= 0 or hi < T:
                if lo >= 0:
                    order.append(lo)
                    lo -= 1
                if hi < T:
                    order.append(hi)
                    hi += 1
        else:
            order = list(range(T))
        if t_lim != T:
            order = list(range(t_lim))
        pair_base = [sum((tt + 2) // 2 for tt in range(t)) for t in range(T)]
        pending = []
        for k, t in enumerate(order):
            need_hps(t)
            if k + 2 < len(order):
                need_hps(order[k + 2])
            npr = (t + 2) // 2
            idx = pair_base[t]
            ws = wpool.tile([TK, NU * 256], FP8, tag="wstrip")
            nc.sync.dma_start(
                ws[:, 0 : npr * 256],
                wt_e[idx : idx + npr].rearrange("n p r c -> p n r c"),
            )
            Ps = [
                pP.tile([128, 512], FP32, tag=f"P{mc}", name=f"Pt{mc}")
                for mc in range(2)
            ]
            for u in range(npr):
                lhsT = ws[:, u * 256 : (u + 1) * 256].rearrange(
                    "p (r c) -> p r c", r=2
                )
                for mc in range(2):
                    nc.tensor.matmul(
                        Ps[mc][:],
                        lhsT,
                        hps[u][:, :, mc * 512 : (mc + 1) * 512],
                        start=(u == 0),
                        stop=(u == npr - 1),
                        perf_mode=mybir.MatmulPerfMode.DoubleRow,
                    )
            # flush the previous tile's deferred Z-path: its Z-matmul now
            # queues BEHIND this tile's main chain on the (FIFO) TensorE, so
            # the Exp it depends on has a whole chain's time to complete
            # instead of head-of-line blocking the next accumulation chain.
            for pt, pEts in pending:
                for mc in range(2):
                    sl = slice(mc * 512, (mc + 1) * 512)
                    pz = pZ.tile([6, 512], FP32, tag=f"Z{mc}", name=f"pz{mc}")
                    nc.tensor.matmul(pz[:], on6[:], pEts[mc][:], start=True, stop=True)
                    lz = ep.tile([6, 512], FP32, tag="lz")
                    nc.scalar.activation(lz[:], pz[:], AF.Ln)
                    rows = 4 if pt == T - 1 else 6
                    nc.vector.tensor_tensor(
                        accZ[0:rows, sl], accZ[0:rows, sl], lz[0:rows, :], op=OP.add
                    )
            pending.clear()
            Ets = []
            for mc in range(2):
                sl = slice(mc * 512, (mc + 1) * 512)
                Et = ep.tile([TK, 512], BF, tag="E", bufs=4, name="Et")
                nc.scalar.activation(
                    Et[:],
                    Ps[mc][0:TK, :],
                    AF.Exp,
                    bias=htt[:, t : t + 1],
                    scale=1.0 / SCALE,
                )
                Ets.append(Et)
                sv = ep.tile([TK, 512], BF, tag="sv")
                nc.vector.tensor_tensor(
                    sv[:], Ps[mc][0:TK, :], hps[t // 2][:, t % 2, sl], op=OP.mult
                )
                nc.vector.tensor_tensor(accS[:, sl], accS[:, sl], sv[:], op=OP.add)
            pending.append((t, Ets))

        for pt, pEts in pending:
            for mc in range(2):
                sl = slice(mc * 512, (mc + 1) * 512)
                pz = pZ.tile([6, 512], FP32, tag=f"Z{mc}", name=f"pzf{mc}")
                nc.tensor.matmul(pz[:], on6[:], pEts[mc][:], start=True, stop=True)
                lz = ep.tile([6, 512], FP32, tag="lz")
                nc.scalar.activation(lz[:], pz[:], AF.Ln)
                rows = 4 if pt == T - 1 else 6
                nc.vector.tensor_tensor(
                    accZ[0:rows, sl], accZ[0:rows, sl], lz[0:rows, :], op=OP.add
                )
        pending.clear()

        # ---- phase D: final reductions ----
        ot = accp.tile([1, 4], FP32)
        nc.vector.memset(ot[:], 0.0)
        dv = accp.tile([1, MS], FP32)
        for mc in range(2):
            sl = slice(mc * 512, (mc + 1) * 512)
            ps_sel = pP.tile([1, 512], FP32, tag="P0", name="ps_sel")
            nc.tensor.matmul(ps_sel[:], o126[:], accS[:, sl], start=True, stop=True)
            ps_z = pZ.tile([1, 512], FP32, tag="Z0", name="ps_z")
            nc.tensor.matmul(ps_z[:], o6f[:], accZ[:, sl], start=True, stop=True)
            d1 = ep.tile([1, 512], FP32, tag="d1")
            nc.vector.tensor_scalar(
                d1[:], ps_sel[:], 1.0 / SCALE, None, OP.mult
            )
            d2 = ep.tile([1, 512], FP32, tag="d2")
            nc.vector.tensor_tensor(d2[:], d1[:], hv[:, sl], op=OP.add)
            nc.vector.tensor_tensor(dv[:, sl], d2[:], ps_z[:], op=OP.subtract)
        uw = accp.tile([1, MS], FP32)
        nc.vector.tensor_tensor(uw[:], dv[:], wv[:], op=OP.mult)
        nc.vector.tensor_reduce(ot[:, 0:1], uw[:], axis=mybir.AxisListType.X, op=OP.add)
        nc.vector.tensor_reduce(
            ot[:, 1:2], wv[:], axis=mybir.AxisListType.X, op=OP.add
        )
        nc.sync.dma_start(out_e[:], ot[:])

    _legalize_waits(nc)
    return nc


_NC_CACHE = None
_CONST_CACHE = None


def _get_nc():
    global _NC_CACHE
    if _NC_CACHE is None:
        _NC_CACHE = build_nc()
    return _NC_CACHE


def _prep_consts():
    global _CONST_CACHE
    if _CONST_CACHE is None:
        p = np.arange(TK)
        _CONST_CACHE = {
            "on6": (p[:, None] // Q == np.arange(6)[None, :]).astype(BF16),
            "o6f": np.ones((6, 1), np.float32),
            "o126": np.ones((TK, 1), np.float32),
        }
    return _CONST_CACHE


def _prep_inputs(seqs, weights, h, J):
    seqs = np.asarray(seqs)
    weights = np.ascontiguousarray(np.asarray(weights, dtype=np.float32))
    h = np.asarray(h, dtype=np.float32)
    J = np.asarray(J, dtype=np.float32)

    seqs32 = seqs.astype(np.int64)
    sqT = np.full((LT, M), Q, dtype=np.float32)
    sqT[:L] = seqs.T.astype(np.float32)
    kcol = (np.arange(JK) % Q).astype(np.float32)
    oh = np.repeat(sqT, Q, axis=0) == kcol[:, None]  # (JK, M) bool
    ohpad = np.zeros((NU * 2 * TK, M), dtype=bool)
    ohpad[:JK] = oh
    ohp = np.ascontiguousarray(
        ohpad.reshape(NU, 2, TK, M).transpose(0, 2, 1, 3)
    ).astype(F8)  # (NU, TK, 2, M)

    Wfull = np.zeros((JK, JK), dtype=np.float32)
    Wfull[: L * Q, : L * Q] = J.transpose(1, 3, 0, 2).reshape(L * Q, L * Q)
    mask126 = np.kron(
        np.triu(np.ones((6, 6), np.float32), 1), np.ones((Q, Q), np.float32)
    )
    wt = np.zeros((NPAIRS, TK, 2, 128), dtype=F8)
    idx = 0
    for t in range(T):
        blockcol = Wfull[:, t * TK : (t + 1) * TK]
        for u in range((t + 2) // 2):
            for r in range(2):
                jt = 2 * u + r
                if jt > t:
                    continue
                tilef = blockcol[jt * TK : (jt + 1) * TK]
                if jt == t:
                    tilef = tilef * mask126
                wt[idx, :, r, :TK] = (tilef * SCALE).astype(F8)
            idx += 1

    hpad = np.zeros(JK, dtype=np.float32)
    hpad[: L * Q] = h.reshape(-1)
    ht_tiles = np.ascontiguousarray(hpad.reshape(T, TK))

    # exact f32 h-gather term: hv[b] = sum_i h[i, seqs[b, i]]
    hsel = h[np.arange(L)[None, :], seqs32].sum(axis=1).astype(np.float32)  # (M,)

    j2 = (J.astype(np.float64) ** 2).sum(axis=(2, 3))
    sumW2 = float((j2 * np.tril(np.ones((L, L)), k=-1)).sum())
    sumh2 = float((h.astype(np.float64) ** 2).sum())

    consts = _prep_consts()
    in_maps = []
    for c in range(NC):
        in_maps.append(
            {
                "wt": wt,
                "hp": np.ascontiguousarray(ohp[..., c * MS : (c + 1) * MS]),
                "ht": ht_tiles,
                "w": weights[c * MS : (c + 1) * MS].reshape(1, MS),
                "hv": hsel[c * MS : (c + 1) * MS].reshape(1, MS),
                **consts,
            }
        )
    return in_maps, (sumW2, sumh2)


def _combine(results, regsums):
    parts = np.stack([np.asarray(r["out"][0]) for r in results])  # (8, 4)
    U = float(parts[:, 0].sum())
    Wsum = float(parts[:, 1].sum())
    nll = -U / max(Wsum, 1e-12)
    sumW2, sumh2 = regsums
    reg = 0.5 * LAMBDA_J * sumW2 + 0.5 * LAMBDA_H * sumh2
    loss = nll + reg
    return (
        np.float32(loss),
        np.float32(nll),
        np.float32(reg),
    )


def kernel(seqs, weights, h, J):
    nc = _get_nc()
    in_maps, regsums = _prep_inputs(seqs, weights, h, J)
    res = run_bass_kernel_spmd(nc, in_maps, core_ids=list(range(NC)))
    return _combine(res.results, regsums)


if __name__ == "__main__":
    d = np.load("/tmp/ref_data.npz")
    out = kernel(d["seqs"], d["weights"], d["h"], d["J"])
    print("kernel:", out)
    print("ref   :", d["loss"], d["nll"], d["reg"])



# revision 8
# speedup vs baseline: 1.0871x; 1.0871x over previous
"""ArDCA loss on 8 TRN2 NeuronCores, data-parallel over M.

Per core (M/8 = 1024 sequences), TK=128 tiling (5376 = 42*128, no padding):
  - P^T = W^T @ H^T as causal fp8 DoubleRow matmuls on TensorE (W scaled x16,
    pre-transposed/masked/packed flat on host; one resident SBUF copy,
    per-tile strip DMAs)
  - Z path: Et = fp8(exp(P/16 + h)) on ScalarE; per-position partition sums
    accumulated across the whole kernel in 4 persistent PSUM banks via
    one-hot DoubleRow matmuls; a single Ln + w-weighted reduce at the end
  - sel path: DVE tensor_tensor_reduce of P against the w-scaled one-hot,
    chained per-partition accumulator (no big adds, no accS tensor)
  - warm-up + bridge dummy matmuls keep TensorE continuously busy so the
    PE p-state ramps to 2.4 GHz and never resets
Host combines the two per-core scalars with exact-f32 h-gather and the
regularizer sums.
"""

import os
import sys

for _p in ("/opt/trn_rl_repo",):
    if _p not in sys.path:
        sys.path.append(_p)

from contextlib import ExitStack

import numpy as np
import ml_dtypes

import concourse.bass as bass
import concourse.mybir as mybir
from concourse import tile
from concourse.bass_utils import run_bass_kernel_spmd

BF16 = ml_dtypes.bfloat16
F8 = ml_dtypes.float8_e4m3
FP32 = mybir.dt.float32
BF = mybir.dt.bfloat16
FP8 = mybir.dt.float8e4

L, Q, M, NC = 256, 21, 8192, 8
LQ = L * Q  # 5376 = 42 * 128
TK, T, NU = 128, 42, 21
MS = M // NC  # 1024
NPR = [(t + 2) // 2 for t in range(T)]
PB = [sum(NPR[:t]) for t in range(T)]
NPAIRS = sum(NPR)  # 462
SCALE = 16.0
LAMBDA_H, LAMBDA_J = 1e-06, 1e-4
AF = mybir.ActivationFunctionType
OP = mybir.AluOpType
DR = mybir.MatmulPerfMode.DoubleRow

# Z-matmul plan: Et pair u2 covers tiles (2u2, 2u2+1); positions 0..127 live
# in tiles 0..20 (2688 rows = 21*128), positions 128..255 in tiles 21..41.
# Pair u2=10 straddles -> two matmuls with one r-slot zeroed each.
# Entries: (u2, bank, zeroed_r or None)
ZPLAN = []
for _u2 in range(NU):
    _t0, _t1 = 2 * _u2, 2 * _u2 + 1
    _b0 = 0 if _t0 <= 20 else 1
    _b1 = 0 if _t1 <= 20 else 1
    if _b0 == _b1:
        ZPLAN.append((_u2, _b0, None))
    else:
        ZPLAN.append((_u2, 0, 1))
        ZPLAN.append((_u2, 1, 0))
NZ = len(ZPLAN)  # 22
ZTOT = {b: sum(1 for (_, bb, _) in ZPLAN if bb == b) for b in (0, 1)}


def _legalize_waits(nc):
    """Split >cap sync waits into preceding EventSemaphore instructions.

    This container's walrus accepts at most 1 wait per instruction (2 on
    EventSemaphore); Tile's final drain carries one wait per used processor.
    """
    n_split = 0
    for f in nc.m.functions:
        for bb in f.blocks:
            if not any(
                ins.sync_info
                and ins.sync_info.on_wait
                and len(ins.sync_info.on_wait)
                > (2 if isinstance(ins, mybir.InstEventSemaphore) else 1)
                for ins in bb.instructions
            ):
                continue
            new_list = []
            for ins in bb.instructions:
                si = ins.sync_info
                waits = list(si.on_wait) if si and si.on_wait else []
                cap = 2 if isinstance(ins, mybir.InstEventSemaphore) else 1
                if len(waits) > cap:
                    extra, keep = waits[:-cap], waits[-cap:]
                    for k in range(0, len(extra), 2):
                        ev = mybir.InstEventSemaphore(
                            name=f"EVSPLIT-{n_split}", ins=[], outs=[]
                        )
                        n_split += 1
                        ev.engine = ins.engine
                        ev.sync_info = mybir.SyncInfo(
                            on_wait=extra[k : k + 2], on_update=[]
                        )
                        new_list.append(ev)
                        nc.register_instruction(ev, overwrite=True)
                    si.on_wait = keep
                new_list.append(ins)
            try:
                bb.instructions = new_list
            except Exception:
                bb.instructions.clear()
                bb.instructions.extend(new_list)
    return n_split


def build_nc():
    nc = bass.Bass()
    wt_e = nc.declare_dram_parameter("wt", [TK, NPAIRS * 256], FP8, isOutput=False)
    hp_e = nc.declare_dram_parameter("hp", [NU, TK, 2, MS], FP8, isOutput=False)
    wh_e = nc.declare_dram_parameter("wh", [T, TK, MS], FP8, isOutput=False)
    ht_e = nc.declare_dram_parameter("ht", [TK, T], FP32, isOutput=False)
    wb_e = nc.declare_dram_parameter("wb", [TK, MS], FP32, isOutput=False)
    oz_e = nc.declare_dram_parameter("oz", [TK, NZ, 2, TK], FP8, isOutput=False)
    o1_e = nc.declare_dram_parameter("o1", [TK, 1], FP32, isOutput=False)
    out_e = nc.declare_dram_parameter("out", [1, 2], FP32, isOutput=True)

    NWARM = int(os.environ.get("KT_WARM", "10"))
    # dummy matmuls inserted before chain t: "t:count,t:count"
    BRD = dict(
        (int(a), int(b))
        for a, b in (
            kv.split(":")
            for kv in os.environ.get("KT_BRIDGE", "2:3,4:1").split(",")
            if kv
        )
    )
    TAILD = int(os.environ.get("KT_TAIL", "4"))
    WH_AHEAD = int(os.environ.get("KT_WHA", "6"))
    WH_BUFS = int(os.environ.get("KT_WHB", "8"))
    ZDELAY = int(os.environ.get("KT_ZDELAY", "2"))

    with tile.TileContext(nc) as tc, ExitStack() as ctx:
        cpool = ctx.enter_context(tc.tile_pool(name="const", bufs=1))
        htp = ctx.enter_context(tc.tile_pool(name="htp", bufs=1))
        whp = ctx.enter_context(tc.tile_pool(name="whp", bufs=1))
        ep = ctx.enter_context(tc.tile_pool(name="ep", bufs=1))
        accp = ctx.enter_context(tc.tile_pool(name="accp", bufs=1))
        pP = ctx.enter_context(tc.tile_pool(name="pP", bufs=2, space="PSUM"))
        pZ = ctx.enter_context(tc.tile_pool(name="pZ", bufs=1, space="PSUM"))

        # ---- constants / resident tensors; per-ring issue order matters ----
        # gpsimd ring carries htt + wh stream + small consts, ordered by
        # first-use time; scalar ring carries hp; sync ring carries W strips.
        whs = {}

        def issue_wh(t):
            if t < T and t not in whs:
                wtile = whp.tile(
                    [TK, MS], FP8, tag="wh", bufs=WH_BUFS, name=f"wh{t}"
                )
                nc.gpsimd.dma_start(wtile[:], wh_e[t])
                whs[t] = wtile

        htt = cpool.tile([TK, T], FP32)
        nc.gpsimd.dma_start(htt[:], ht_e[:])
        issue_wh(0)
        issue_wh(1)
        ozs = cpool.tile([TK, NZ, 2, TK], FP8)
        nc.gpsimd.dma_start(ozs[:], oz_e[:])
        for _t in range(2, WH_AHEAD):
            issue_wh(_t)
        w128 = cpool.tile([TK, MS], FP32)
        nc.gpsimd.dma_start(w128[:], wb_e[:])
        o1 = cpool.tile([TK, 1], FP32)
        nc.gpsimd.dma_start(o1[:], o1_e[:])

        # scalar ring: one-hot pair tiles, in order
        hps = [
            htp.tile([TK, 2, MS], FP8, tag=f"hp{u}", name=f"hp{u}") for u in range(NU)
        ]
        for u in range(NU):
            nc.scalar.dma_start(hps[u][:], hp_e[u])

        # sync ring: W strips (flat resident layout), in tile order
        wts = cpool.tile([TK, NPAIRS * 256], FP8)
        for t in range(T):
            a, b = PB[t] * 256, (PB[t] + NPR[t]) * 256
            nc.sync.dma_start(wts[:, a:b], wt_e[:, a:b])

        # warm-up source + dummy matmuls (keep PE busy, ramp the p-state)
        warm = cpool.tile([TK, 1024], FP8)
        nc.vector.memset(warm[:], 0.0)
        wlhsT = warm[:, 0:256].rearrange("p (r c) -> p r c", r=2)
        wrhs = warm[:].rearrange("p (r c) -> p r c", r=2)
        _di = [0]

        def dummy(n, tags=("za00", "za01", "za10", "za11")):
            for _ in range(n):
                zt = pZ.tile(
                    [TK, 512], FP32, tag=tags[_di[0] % len(tags)], name="zd"
                )
                nc.tensor.matmul(
                    zt[:], wlhsT, wrhs, start=True, stop=True, perf_mode=DR
                )
                _di[0] += 1

        dummy(NWARM)

        # persistent accumulators: striped per-(tile,mc) partial columns,
        # reduced once at the end (no serial chains)
        accSel = accp.tile([TK, 2 * T], FP32)
        accZw = accp.tile([TK, 4], FP32)
        zs2 = accp.tile([TK, 2], FP32)

        # Et pair buffers + Zacc banks
        ets = {}
        zacc = {}
        zemit = {0: 0, 1: 0}

        def emit_z(u2):
            etc = ets[u2]
            for z, (u2_, b, zr) in enumerate(ZPLAN):
                if u2_ != u2:
                    continue
                first = zemit[b] == 0
                zemit[b] += 1
                last = zemit[b] == ZTOT[b]
                for mc in range(2):
                    key = (b, mc)
                    if key not in zacc:
                        zacc[key] = pZ.tile(
                            [TK, 512], FP32, tag=f"za{b}{mc}", name=f"za{b}{mc}"
                        )
                    nc.tensor.matmul(
                        zacc[key][:],
                        ozs[:, z],
                        etc[mc][:],
                        start=first,
                        stop=last,
                        perf_mode=DR,
                    )
                if last:
                    for mc in range(2):
                        lz = ep.tile([TK, 512], BF, tag="lz", bufs=2, name="lz")
                        nc.scalar.activation(lz[:], zacc[(b, mc)][:], AF.Ln)
                        zc = ep.tile([TK, 512], BF, tag="zc", bufs=2, name="zc")
                        nc.vector.tensor_tensor(
                            zc[:],
                            lz[:],
                            w128[:, mc * 512 : (mc + 1) * 512],
                            op=OP.mult,
                        )
                        col = 2 * b + mc
                        nc.vector.tensor_reduce(
                            accZw[:, col : col + 1],
                            zc[:],
                            axis=mybir.AxisListType.X,
                            op=OP.add,
                        )

        # ---- main causal loop ----
        next_z = 0
        for t in range(T):
            issue_wh(t + WH_AHEAD)
            if t in BRD:
                # bank0's accumulation group opens at t=3; bank1's only at
                # t=23 — route mid-loop dummies to still-closed bank1 banks
                dummy(
                    BRD[t],
                    tags=(
                        ("za10", "za11")
                        if t >= 3
                        else ("za00", "za01", "za10", "za11")
                    ),
                )
            npr = NPR[t]
            Ps = [
                pP.tile([TK, 512], FP32, tag=f"P{mc}", name=f"P{mc}")
                for mc in range(2)
            ]
            for u in range(npr):
                a = (PB[t] + u) * 256
                lhsT = wts[:, a : a + 256].rearrange("p (r c) -> p r c", r=2)
                for mc in range(2):
                    nc.tensor.matmul(
                        Ps[mc][:],
                        lhsT,
                        hps[u][:, :, mc * 512 : (mc + 1) * 512],
                        start=(u == 0),
                        stop=(u == npr - 1),
                        perf_mode=DR,
                    )
            # deferred Z matmuls: pair u2 complete at tile 2u2+1, give the
            # Exps ZDELAY chains of slack before queueing behind this chain
            while next_z < NU and 2 * next_z + 1 <= t - ZDELAY:
                emit_z(next_z)
                next_z += 1
            # consumers of this chain
            u2c = t // 2
            if u2c not in ets:
                ets[u2c] = [
                    ep.tile([TK, 2, 512], FP8, tag=f"E{mc}", bufs=3, name=f"et{mc}")
                    for mc in range(2)
                ]
            for mc in range(2):
                nc.scalar.activation(
                    ets[u2c][mc][:, t % 2, :],
                    Ps[mc][:],
                    AF.Exp,
                    bias=htt[:, t : t + 1],
                    scale=1.0 / SCALE,
                )
            for mc in range(2):
                sc = ep.tile([TK, 512], BF, tag="sc", bufs=2, name="sc")
                nc.vector.tensor_tensor(
                    sc[:],
                    Ps[mc][:],
                    whs[t][:, mc * 512 : (mc + 1) * 512],
                    op=OP.mult,
                )
                col = 2 * t + mc
                nc.vector.tensor_reduce(
                    accSel[:, col : col + 1],
                    sc[:],
                    axis=mybir.AxisListType.X,
                    op=OP.add,
                )

        # ---- tail: flush remaining Z pairs, final reduce ----
        while next_z < NU:
            if next_z == NU - 1:
                dummy(TAILD, tags=("za00", "za01"))
            emit_z(next_z)
            next_z += 1

        nc.vector.tensor_reduce(
            zs2[:, 1:2], accSel[:], axis=mybir.AxisListType.X, op=OP.add
        )
        nc.vector.tensor_reduce(
            zs2[:, 0:1], accZw[:], axis=mybir.AxisListType.X, op=OP.add
        )
        pfin = pP.tile([1, 2], FP32, tag="P0", name="pfin")
        nc.tensor.matmul(pfin[:], o1[:], zs2[:], start=True, stop=True)
        ot = accp.tile([1, 2], FP32)
        nc.scalar.copy(ot[:], pfin[:])
        nc.sync.dma_start(out_e[:], ot[:])

    _legalize_waits(nc)
    return nc


_NC_CACHE = None
_CONST_CACHE = None


def _get_nc():
    global _NC_CACHE
    if _NC_CACHE is None:
        _NC_CACHE = build_nc()
    return _NC_CACHE


def _prep_consts():
    global _CONST_CACHE
    if _CONST_CACHE is None:
        oz = np.zeros((TK, NZ, 2, TK), dtype=F8)
        p = np.arange(TK)
        for z, (u2, b, zr) in enumerate(ZPLAN):
            for r in range(2):
                if zr == r:
                    continue
                pos = (256 * u2 + 128 * r + p) // Q
                c = pos - 128 * b
                oz[p, z, r, c] = 1.0
        _CONST_CACHE = {
            "oz": oz,
            "o1": np.ones((TK, 1), np.float32),
        }
    return _CONST_CACHE


def _prep_inputs(seqs, weights, h, J):
    seqs = np.asarray(seqs)
    weights = np.ascontiguousarray(np.asarray(weights, dtype=np.float32))
    h = np.asarray(h, dtype=np.float32)
    J = np.asarray(J, dtype=np.float32)

    # W[jk, ia] = J[i, j, a, k], masked to pos(j) < pos(i), x16, fp8
    W = J.transpose(1, 3, 0, 2).reshape(LQ, LQ)
    pos = np.arange(LQ) // Q
    W8 = np.where(pos[:, None] < pos[None, :], W * SCALE, 0.0).astype(F8)
    W8v = W8.reshape(T, TK, T, TK)  # [jt, p, t, c]

    strips = []
    for t in range(T):
        blk = W8v[0 : 2 * NPR[t], :, t, :]  # [2npr, p, c]
        strips.append(
            blk.reshape(NPR[t], 2, TK, TK).transpose(2, 0, 1, 3).reshape(TK, -1)
        )
    wt = np.ascontiguousarray(np.concatenate(strips, axis=1))  # [TK, NPAIRS*256]

    # one-hot H^T (LQ, M)
    s32 = seqs.astype(np.int32)
    ohb = s32.T.repeat(Q, axis=0) == (np.arange(LQ, dtype=np.int32) % Q)[:, None]
    oh8 = ohb.astype(F8)
    hp = oh8.reshape(NU, 2, TK, M).transpose(0, 2, 1, 3)  # [NU, TK, 2, M]
    wh = (ohb * weights[None, :]).astype(F8).reshape(T, TK, M)

    ht = np.ascontiguousarray(h.reshape(T, TK).T)  # [TK, T]

    # exact host-side pieces (f64)
    hsel = h[np.arange(L)[None, :], s32].sum(axis=1).astype(np.float64)  # (M,)
    w64 = weights.astype(np.float64)
    hsel_w = float((hsel * w64).sum())
    wsum = float(w64.sum())
    j2 = (J.astype(np.float64) ** 2).sum(axis=(2, 3))
    sumW2 = float((j2 * np.tril(np.ones((L, L)), k=-1)).sum())
    sumh2 = float((h.astype(np.float64) ** 2).sum())

    consts = _prep_consts()
    in_maps = []
    for c in range(NC):
        sl = slice(c * MS, (c + 1) * MS)
        in_maps.append(
            {
                "wt": wt,
                "hp": np.ascontiguousarray(hp[..., sl]),
                "wh": np.ascontiguousarray(wh[..., sl]),
                "ht": ht,
                "wb": np.ascontiguousarray(
                    np.broadcast_to(weights[sl][None, :], (TK, MS))
                ),
                **consts,
            }
        )
    return in_maps, (hsel_w, wsum, sumW2, sumh2)


def _combine(results, hostsums):
    parts = np.stack([np.asarray(r["out"][0]) for r in results])  # (8, 2)
    Zw = float(parts[:, 0].sum())
    Uw = float(parts[:, 1].sum())
    hsel_w, wsum, sumW2, sumh2 = hostsums
    nll = (Zw - Uw / SCALE - hsel_w) / max(wsum, 1e-12)
    reg = 0.5 * LAMBDA_J * sumW2 + 0.5 * LAMBDA_H * sumh2
    loss = nll + reg
    return (
        np.float32(loss),
        np.float32(nll),
        np.float32(reg),
    )


def kernel(seqs, weights, h, J):
    nc = _get_nc()
    in_maps, hostsums = _prep_inputs(seqs, weights, h, J)
    res = run_bass_kernel_spmd(nc, in_maps, core_ids=list(range(NC)))
    return _combine(res.results, hostsums)


if __name__ == "__main__":
    d = np.load("/tmp/ref_data.npz")
    out = kernel(d["seqs"], d["weights"], d["h"], d["J"])
    print("kernel:", out)
    print("ref   :", d["loss"], d["nll"], d["reg"])


# revision 13
# speedup vs baseline: 1.1362x; 1.0451x over previous
"""ArDCA loss on 8 TRN2 NeuronCores, data-parallel over M.

Per core (M/8 = 1024 sequences), TK=128 tiling (5376 = 42*128, no padding):
  - P^T = W^T @ H^T as causal fp8 DoubleRow matmuls on TensorE (W scaled x16,
    pre-transposed/masked/packed flat on host; one resident SBUF copy,
    per-tile strip DMAs)
  - Z path: Et = fp8(exp(P/16 + h)) on ScalarE; per-position partition sums
    accumulated across the whole kernel in 4 persistent PSUM banks via
    one-hot DoubleRow matmuls; a single Ln + w-weighted reduce at the end
  - sel path: DVE tensor_tensor_reduce of P against the w-scaled one-hot,
    chained per-partition accumulator (no big adds, no accS tensor)
  - warm-up + bridge dummy matmuls keep TensorE continuously busy so the
    PE p-state ramps to 2.4 GHz and never resets
Host combines the two per-core scalars with exact-f32 h-gather and the
regularizer sums.
"""

import os
import sys

for _p in ("/opt/trn_rl_repo",):
    if _p not in sys.path:
        sys.path.append(_p)

from contextlib import ExitStack

import numpy as np
import ml_dtypes

import concourse.bass as bass
import concourse.mybir as mybir
from concourse import tile
from concourse.bass_utils import run_bass_kernel_spmd

BF16 = ml_dtypes.bfloat16
F8 = ml_dtypes.float8_e4m3
FP32 = mybir.dt.float32
BF = mybir.dt.bfloat16
FP8 = mybir.dt.float8e4

L, Q, M, NC = 256, 21, 8192, 8
LQ = L * Q  # 5376 = 42 * 128
TK, T, NU = 128, 42, 21
MS = M // NC  # 1024
NPR = [(t + 2) // 2 for t in range(T)]
PB = [sum(NPR[:t]) for t in range(T)]
NPAIRS = sum(NPR)  # 462
SCALE = 16.0
LAMBDA_H, LAMBDA_J = 1e-06, 1e-4
AF = mybir.ActivationFunctionType
OP = mybir.AluOpType
DR = mybir.MatmulPerfMode.DoubleRow

# Z-matmul plan: Et pair u2 covers tiles (2u2, 2u2+1); positions 0..127 live
# in tiles 0..20 (2688 rows = 21*128), positions 128..255 in tiles 21..41.
# Pair u2=10 straddles -> two matmuls with one r-slot zeroed each.
# Entries: (u2, bank, zeroed_r or None)
ZPLAN = []
for _u2 in range(NU):
    _t0, _t1 = 2 * _u2, 2 * _u2 + 1
    _b0 = 0 if _t0 <= 20 else 1
    _b1 = 0 if _t1 <= 20 else 1
    if _b0 == _b1:
        ZPLAN.append((_u2, _b0, None))
    else:
        ZPLAN.append((_u2, 0, 1))
        ZPLAN.append((_u2, 1, 0))
NZ = len(ZPLAN)  # 22
ZTOT = {b: sum(1 for (_, bb, _) in ZPLAN if bb == b) for b in (0, 1)}


def _legalize_waits(nc):
    """Split >cap sync waits into preceding EventSemaphore instructions.

    This container's walrus accepts at most 1 wait per instruction (2 on
    EventSemaphore); Tile's final drain carries one wait per used processor.
    """
    n_split = 0
    for f in nc.m.functions:
        for bb in f.blocks:
            if not any(
                ins.sync_info
                and ins.sync_info.on_wait
                and len(ins.sync_info.on_wait)
                > (2 if isinstance(ins, mybir.InstEventSemaphore) else 1)
                for ins in bb.instructions
            ):
                continue
            new_list = []
            for ins in bb.instructions:
                si = ins.sync_info
                waits = list(si.on_wait) if si and si.on_wait else []
                cap = 2 if isinstance(ins, mybir.InstEventSemaphore) else 1
                if len(waits) > cap:
                    extra, keep = waits[:-cap], waits[-cap:]
                    for k in range(0, len(extra), 2):
                        ev = mybir.InstEventSemaphore(
                            name=f"EVSPLIT-{n_split}", ins=[], outs=[]
                        )
                        n_split += 1
                        ev.engine = ins.engine
                        ev.sync_info = mybir.SyncInfo(
                            on_wait=extra[k : k + 2], on_update=[]
                        )
                        new_list.append(ev)
                        nc.register_instruction(ev, overwrite=True)
                    si.on_wait = keep
                new_list.append(ins)
            try:
                bb.instructions = new_list
            except Exception:
                bb.instructions.clear()
                bb.instructions.extend(new_list)
    return n_split


def build_nc():
    nc = bass.Bass()
    wt_e = nc.declare_dram_parameter("wt", [TK, NPAIRS * 256], FP8, isOutput=False)
    hp_e = nc.declare_dram_parameter("hp", [NU, TK, 2, MS], FP8, isOutput=False)
    wh_e = nc.declare_dram_parameter("wh", [T, TK, MS], FP8, isOutput=False)
    ht_e = nc.declare_dram_parameter("ht", [TK, T], FP32, isOutput=False)
    wb_e = nc.declare_dram_parameter("wb", [TK, MS], FP32, isOutput=False)
    oz_e = nc.declare_dram_parameter("oz", [TK, NZ, 2, TK], FP8, isOutput=False)
    o1_e = nc.declare_dram_parameter("o1", [TK, 1], FP32, isOutput=False)
    out_e = nc.declare_dram_parameter("out", [1, 2], FP32, isOutput=True)

    NWARM = int(os.environ.get("KT_WARM", "8"))
    # dummy matmuls inserted before chain t: "t:count,t:count"
    BRD = dict(
        (int(a), int(b))
        for a, b in (
            kv.split(":")
            for kv in os.environ.get("KT_BRIDGE", "2:3,4:1").split(",")
            if kv
        )
    )
    TAILD = int(os.environ.get("KT_TAIL", "4"))
    WH_AHEAD = int(os.environ.get("KT_WHA", "6"))
    WH_BUFS = int(os.environ.get("KT_WHB", "8"))
    ZDELAY = int(os.environ.get("KT_ZDELAY", "2"))

    with tile.TileContext(nc) as tc, ExitStack() as ctx:
        cpool = ctx.enter_context(tc.tile_pool(name="const", bufs=1))
        htp = ctx.enter_context(tc.tile_pool(name="htp", bufs=1))
        whp = ctx.enter_context(tc.tile_pool(name="whp", bufs=1))
        ep = ctx.enter_context(tc.tile_pool(name="ep", bufs=1))
        accp = ctx.enter_context(tc.tile_pool(name="accp", bufs=1))
        pP = ctx.enter_context(tc.tile_pool(name="pP", bufs=2, space="PSUM"))
        pZ = ctx.enter_context(tc.tile_pool(name="pZ", bufs=1, space="PSUM"))

        # ---- constants / resident tensors; per-ring issue order matters ----
        # gpsimd ring carries htt + wh stream + small consts, ordered by
        # first-use time; scalar ring carries hp; sync ring carries W strips.
        whs = {}

        def issue_wh(t):
            if t < T and t not in whs:
                wtile = whp.tile(
                    [TK, MS], FP8, tag="wh", bufs=WH_BUFS, name=f"wh{t}"
                )
                nc.gpsimd.dma_start(wtile[:], wh_e[t])
                whs[t] = wtile

        htt = cpool.tile([TK, T], FP32)
        nc.gpsimd.dma_start(htt[:], ht_e[:])
        issue_wh(0)
        issue_wh(1)
        ozs = cpool.tile([TK, NZ, 2, TK], FP8)
        nc.gpsimd.dma_start(ozs[:], oz_e[:])
        for _t in range(2, WH_AHEAD):
            issue_wh(_t)
        w128 = cpool.tile([TK, MS], FP32)
        nc.gpsimd.dma_start(w128[:], wb_e[:])
        o1 = cpool.tile([TK, 1], FP32)
        nc.gpsimd.dma_start(o1[:], o1_e[:])

        # scalar ring: one-hot pair tiles, issued JIT (a few ahead) so early
        # DMA-engine bandwidth goes to the tiles the PE needs first
        hps = [
            htp.tile([TK, 2, MS], FP8, tag=f"hp{u}", name=f"hp{u}") for u in range(NU)
        ]
        hp_issued = set()

        def issue_hp(u):
            if u < NU and u not in hp_issued:
                hp_issued.add(u)
                nc.scalar.dma_start(hps[u][:], hp_e[u])

        HP_AHEAD = int(os.environ.get("KT_HPA", "5"))
        for _u in range(HP_AHEAD):
            issue_hp(_u)

        # sync ring: W strips — one tile per output tile (separate tiles keep
        # per-strip DMA dependencies; a single resident tile would make the
        # first matmul wait on every strip), issued JIT
        strips = [
            cpool.tile([TK, NPR[t] * 256], FP8, name=f"wts{t}") for t in range(T)
        ]
        st_issued = set()

        def issue_strip(t):
            if t < T and t not in st_issued:
                st_issued.add(t)
                a, b = PB[t] * 256, (PB[t] + NPR[t]) * 256
                nc.sync.dma_start(strips[t][:], wt_e[:, a:b])

        ST_AHEAD = int(os.environ.get("KT_STA", "6"))
        for _t in range(ST_AHEAD):
            issue_strip(_t)

        # warm-up source + dummy matmuls (keep PE busy, ramp the p-state)
        warm = cpool.tile([TK, 1024], FP8)
        nc.vector.memset(warm[:], 0.0)
        wlhsT = warm[:, 0:256].rearrange("p (r c) -> p r c", r=2)
        wrhs = warm[:].rearrange("p (r c) -> p r c", r=2)
        _di = [0]

        def dummy(n, tags=("za00", "za01", "za10", "za11")):
            for _ in range(n):
                zt = pZ.tile(
                    [TK, 512], FP32, tag=tags[_di[0] % len(tags)], name="zd"
                )
                nc.tensor.matmul(
                    zt[:], wlhsT, wrhs, start=True, stop=True, perf_mode=DR
                )
                _di[0] += 1

        dummy(NWARM)

        # persistent accumulators: striped per-(tile,mc) partial columns,
        # reduced once at the end (no serial chains)
        accSel = accp.tile([TK, 2 * T], FP32)
        accZw = accp.tile([TK, 4], FP32)
        zs2 = accp.tile([TK, 2], FP32)

        # Et pair buffers + Zacc banks
        ets = {}
        zacc = {}
        zemit = {0: 0, 1: 0}

        def emit_z(u2):
            etc = ets[u2]
            for z, (u2_, b, zr) in enumerate(ZPLAN):
                if u2_ != u2:
                    continue
                first = zemit[b] == 0
                zemit[b] += 1
                last = zemit[b] == ZTOT[b]
                for mc in range(2):
                    key = (b, mc)
                    if key not in zacc:
                        zacc[key] = pZ.tile(
                            [TK, 512], FP32, tag=f"za{b}{mc}", name=f"za{b}{mc}"
                        )
                    nc.tensor.matmul(
                        zacc[key][:],
                        ozs[:, z],
                        etc[mc][:],
                        start=first,
                        stop=last,
                        perf_mode=DR,
                    )
                if last:
                    for mc in range(2):
                        lz = ep.tile([TK, 512], BF, tag="lz", bufs=2, name="lz")
                        nc.scalar.activation(lz[:], zacc[(b, mc)][:], AF.Ln)
                        zc = ep.tile([TK, 512], BF, tag="zc", bufs=2, name="zc")
                        nc.vector.tensor_tensor(
                            zc[:],
                            lz[:],
                            w128[:, mc * 512 : (mc + 1) * 512],
                            op=OP.mult,
                        )
                        col = 2 * b + mc
                        nc.vector.tensor_reduce(
                            accZw[:, col : col + 1],
                            zc[:],
                            axis=mybir.AxisListType.X,
                            op=OP.add,
                        )

        # ---- main causal loop ----
        next_z = 0
        for t in range(T):
            issue_wh(t + WH_AHEAD)
            issue_strip(t + ST_AHEAD)
            for _u in range((t + 2) // 2 + HP_AHEAD):
                issue_hp(_u)
            if t in BRD:
                # bank0's accumulation group opens at t=3; bank1's only at
                # t=23 — route mid-loop dummies to still-closed bank1 banks
                dummy(
                    BRD[t],
                    tags=(
                        ("za10", "za11")
                        if t >= 3
                        else ("za00", "za01", "za10", "za11")
                    ),
                )
            npr = NPR[t]
            Ps = [
                pP.tile([TK, 512], FP32, tag=f"P{mc}", name=f"P{mc}")
                for mc in range(2)
            ]
            for u in range(npr):
                a = u * 256
                lhsT = strips[t][:, a : a + 256].rearrange("p (r c) -> p r c", r=2)
                for mc in range(2):
                    nc.tensor.matmul(
                        Ps[mc][:],
                        lhsT,
                        hps[u][:, :, mc * 512 : (mc + 1) * 512],
                        start=(u == 0),
                        stop=(u == npr - 1),
                        perf_mode=DR,
                    )
            # deferred Z matmuls: pair u2 complete at tile 2u2+1, give the
            # Exps ZDELAY chains of slack before queueing behind this chain
            while next_z < NU and 2 * next_z + 1 <= t - ZDELAY:
                emit_z(next_z)
                next_z += 1
            # consumers of this chain
            u2c = t // 2
            if u2c not in ets:
                ets[u2c] = [
                    ep.tile([TK, 2, 512], FP8, tag=f"E{mc}", bufs=3, name=f"et{mc}")
                    for mc in range(2)
                ]
            for mc in range(2):
                nc.scalar.activation(
                    ets[u2c][mc][:, t % 2, :],
                    Ps[mc][:],
                    AF.Exp,
                    bias=htt[:, t : t + 1],
                    scale=1.0 / SCALE,
                )
            for mc in range(2):
                sc = ep.tile([TK, 512], BF, tag="sc", bufs=2, name="sc")
                nc.vector.tensor_tensor(
                    sc[:],
                    Ps[mc][:],
                    whs[t][:, mc * 512 : (mc + 1) * 512],
                    op=OP.mult,
                )
                col = 2 * t + mc
                nc.vector.tensor_reduce(
                    accSel[:, col : col + 1],
                    sc[:],
                    axis=mybir.AxisListType.X,
                    op=OP.add,
                )

        # ---- tail: flush remaining Z pairs, final reduce ----
        while next_z < NU:
            if next_z == NU - 1:
                dummy(TAILD, tags=("za00", "za01"))
            emit_z(next_z)
            next_z += 1

        nc.vector.tensor_reduce(
            zs2[:, 1:2], accSel[:], axis=mybir.AxisListType.X, op=OP.add
        )
        nc.vector.tensor_reduce(
            zs2[:, 0:1], accZw[:], axis=mybir.AxisListType.X, op=OP.add
        )
        pfin = pP.tile([1, 2], FP32, tag="P0", name="pfin")
        nc.tensor.matmul(pfin[:], o1[:], zs2[:], start=True, stop=True)
        ot = accp.tile([1, 2], FP32)
        nc.scalar.copy(ot[:], pfin[:])
        nc.sync.dma_start(out_e[:], ot[:])

    _legalize_waits(nc)
    return nc


_NC_CACHE = None
_CONST_CACHE = None


def _get_nc():
    global _NC_CACHE
    if _NC_CACHE is None:
        _NC_CACHE = build_nc()
    return _NC_CACHE


def _prep_consts():
    global _CONST_CACHE
    if _CONST_CACHE is None:
        oz = np.zeros((TK, NZ, 2, TK), dtype=F8)
        p = np.arange(TK)
        for z, (u2, b, zr) in enumerate(ZPLAN):
            for r in range(2):
                if zr == r:
                    continue
                pos = (256 * u2 + 128 * r + p) // Q
                c = pos - 128 * b
                oz[p, z, r, c] = 1.0
        _CONST_CACHE = {
            "oz": oz,
            "o1": np.ones((TK, 1), np.float32),
        }
    return _CONST_CACHE


def _prep_inputs(seqs, weights, h, J):
    seqs = np.asarray(seqs)
    weights = np.ascontiguousarray(np.asarray(weights, dtype=np.float32))
    h = np.asarray(h, dtype=np.float32)
    J = np.asarray(J, dtype=np.float32)

    # W[jk, ia] = J[i, j, a, k], masked to pos(j) < pos(i), x16, fp8
    W = J.transpose(1, 3, 0, 2).reshape(LQ, LQ)
    pos = np.arange(LQ) // Q
    W8 = np.where(pos[:, None] < pos[None, :], W * SCALE, 0.0).astype(F8)
    W8v = W8.reshape(T, TK, T, TK)  # [jt, p, t, c]

    strips = []
    for t in range(T):
        blk = W8v[0 : 2 * NPR[t], :, t, :]  # [2npr, p, c]
        strips.append(
            blk.reshape(NPR[t], 2, TK, TK).transpose(2, 0, 1, 3).reshape(TK, -1)
        )
    wt = np.ascontiguousarray(np.concatenate(strips, axis=1))  # [TK, NPAIRS*256]

    # one-hot H^T (LQ, M)
    s32 = seqs.astype(np.int32)
    ohb = s32.T.repeat(Q, axis=0) == (np.arange(LQ, dtype=np.int32) % Q)[:, None]
    oh8 = ohb.astype(F8)
    hp = oh8.reshape(NU, 2, TK, M).transpose(0, 2, 1, 3)  # [NU, TK, 2, M]
    wh = (ohb * weights[None, :]).astype(F8).reshape(T, TK, M)

    ht = np.ascontiguousarray(h.reshape(T, TK).T)  # [TK, T]

    # exact host-side pieces (f64)
    hsel = h[np.arange(L)[None, :], s32].sum(axis=1).astype(np.float64)  # (M,)
    w64 = weights.astype(np.float64)
    hsel_w = float((hsel * w64).sum())
    wsum = float(w64.sum())
    j2 = (J.astype(np.float64) ** 2).sum(axis=(2, 3))
    sumW2 = float((j2 * np.tril(np.ones((L, L)), k=-1)).sum())
    sumh2 = float((h.astype(np.float64) ** 2).sum())

    consts = _prep_consts()
    in_maps = []
    for c in range(NC):
        sl = slice(c * MS, (c + 1) * MS)
        in_maps.append(
            {
                "wt": wt,
                "hp": np.ascontiguousarray(hp[..., sl]),
                "wh": np.ascontiguousarray(wh[..., sl]),
                "ht": ht,
                "wb": np.ascontiguousarray(
                    np.broadcast_to(weights[sl][None, :], (TK, MS))
                ),
                **consts,
            }
        )
    return in_maps, (hsel_w, wsum, sumW2, sumh2)


def _combine(results, hostsums):
    parts = np.stack([np.asarray(r["out"][0]) for r in results])  # (8, 2)
    Zw = float(parts[:, 0].sum())
    Uw = float(parts[:, 1].sum())
    hsel_w, wsum, sumW2, sumh2 = hostsums
    nll = (Zw - Uw / SCALE - hsel_w) / max(wsum, 1e-12)
    reg = 0.5 * LAMBDA_J * sumW2 + 0.5 * LAMBDA_H * sumh2
    loss = nll + reg
    return (
        np.float32(loss),
        np.float32(nll),
        np.float32(reg),
    )


def kernel(seqs, weights, h, J):
    nc = _get_nc()
    in_maps, hostsums = _prep_inputs(seqs, weights, h, J)
    res = run_bass_kernel_spmd(nc, in_maps, core_ids=list(range(NC)))
    return _combine(res.results, hostsums)


if __name__ == "__main__":
    d = np.load("/tmp/ref_data.npz")
    out = kernel(d["seqs"], d["weights"], d["h"], d["J"])
    print("kernel:", out)
    print("ref   :", d["loss"], d["nll"], d["reg"])
